# revision 32
# baseline (speedup 1.0000x reference)
"""Trainium2 Bass kernel for BatchedModelManifoldGeodesicFlow.

Closed-form math (per sample), derived from the reference's autodiff:
  f(x) = tanh(x@W1 + b1)@W2 + b2,  J = jacrev(f)(x) = W2^T diag(d) W1^T
  with h = tanh(x@W1+b1), d = 1-h^2, e = -2*h*d, K = W1^T W1, L = W2 W2^T.
  V := L diag(d) K,  W := K diag(d) L (= V^T),  U := K diag(d) V,
  F := (K.*L).*U + K.*V.*W  (symmetric; uses e = -2*h*d)
    ||dG||^2 = 8 * (h.*d)^T F (h.*d)
  Christoffel contraction -> small matvecs:
    S1 = W2^T [ e.*cv.*w + d.*(K (e.*g.*w)) ],  0.5*S2 = W1 (e.*g.*cv)
    w = W1^T v, g = W2 v, cv = K (d.*g)
    a = (0.5*S2 - S1) / ((||dG||+1e-6) * (||v||+1e-6))
  out = concat([v, a - 0.1*dev], axis=0)

Sharding: pure data parallel, batch 16 -> 2 samples per core on 8 cores.
All heavy matmuls in bf16 (errors cancel statistically in the big norm
sums); the tanh-input matmul stays float32r for accuracy of h.
"""

import sys

if "/opt/trn_rl_repo" not in sys.path:
    sys.path.insert(0, "/opt/trn_rl_repo")

import numpy as np

import concourse.bacc as bacc
import concourse.tile as tile
from concourse import mybir
from concourse.masks import make_identity
from concourse.tile import add_dep_helper

N = 128
H = 256
B = 16
NCORES = 8
BLOC = B // NCORES  # 2 samples per core

F32 = mybir.dt.float32
F32R = mybir.dt.float32r
BF16 = mybir.dt.bfloat16
I32 = mybir.dt.int32
AF = mybir.ActivationFunctionType
OP = mybir.AluOpType
AX = mybir.AxisListType

SQRT_MAGIC = 0x1FBD1DF5  # bits trick: sqrt(x) ~ bitcast((bits(x)>>1) + MAGIC)


def ts(i, sz=128):
    return slice(i * sz, (i + 1) * sz)


def build_nc():
    nc = bacc.Bacc(trn_type="TRN2", enable_partition_id=False)

    # vecs rows: dev(0:2) x0(2:4) x1(4:6) vel(6:8)
    d_vecs = nc.dram_tensor("vecs", [4 * BLOC, N], F32, kind="ExternalInput")
    d_w1 = nc.dram_tensor("W1", [N, H], F32, kind="ExternalInput")
    d_w2 = nc.dram_tensor("W2", [H, N], F32, kind="ExternalInput")
    # misc cols: t128 | b1c (2 cols) | pad
    d_misc = nc.dram_tensor("misc", [128, 4], F32, kind="ExternalInput")
    d_out = nc.dram_tensor("out_bot", [BLOC, N], F32, kind="ExternalOutput")

    with tile.TileContext(nc) as tc:
        with (
            tc.tile_pool(name="consts", bufs=1) as consts,
            tc.tile_pool(name="work", bufs=1) as work,
            tc.tile_pool(name="loop", bufs=2) as loop,
            tc.tile_pool(name="pbig", bufs=4, space="PSUM") as pbig,
            tc.tile_pool(name="psmall", bufs=3, space="PSUM") as psmall,
            tc.tile_pool(name="prow", bufs=1, space="PSUM") as prow,
        ):
            _emit(nc, consts, work, loop, pbig, psmall, prow,
                  d_vecs, d_w1, d_w2, d_misc, d_out)

    nc.compile()  # Bacc: split multi-waits into event semaphores, alloc regs
    return nc


def _emit(nc, consts, work, loop, pbig, psmall, prow,
          d_vecs, d_w1, d_w2, d_misc, d_out):
    # ---------------- input DMA (2 trigger engines in parallel) --------
    sb_w1 = consts.tile([128, H], F32)          # W1[k, m]
    sb_w2 = consts.tile([128, 2, 128], F32)     # W2 tiles: [m%128, m//128, j]
    sb_vecs = consts.tile([4 * BLOC, N], F32)
    sb_misc = consts.tile([128, 4], F32)
    nc.sync.dma_start(out=sb_vecs, in_=d_vecs[:, :])
    nc.sync.dma_start(out=sb_w1, in_=d_w1[:, :])
    nc.sync.dma_start(out=sb_w2, in_=d_w2.rearrange("(t p) n -> p t n", p=128))
    nc.scalar.dma_start(out=sb_misc, in_=d_misc[:, :])
    t128 = sb_misc[:, 0:1]
    sb_b1c = sb_misc[:, 1:3]
    dev_rows = sb_vecs[0:BLOC, :]

    # w1_r early: it gates the tanh-input matmul on the critical path
    w1_r = consts.tile([128, H], F32R)
    nc.vector.tensor_copy(w1_r, sb_w1)

    # ---------------- constants ----------------
    ident = consts.tile([128, 128], F32)
    make_identity(nc, ident)
    ones2 = consts.tile([128, 2], F32)
    nc.vector.memset(ones2, 1.0)
    ones2_r = consts.tile([128, 2], F32R)
    nc.vector.tensor_copy(ones2_r, ones2)

    # ---------------- t window: wf4 = 4t(1-t) ----------------
    omt = work.tile([128, 1], F32)
    nc.vector.tensor_scalar(
        out=omt, in0=t128, scalar1=-1.0, scalar2=1.0, op0=OP.mult, op1=OP.add
    )
    wf4 = work.tile([128, 1], F32)
    nc.vector.tensor_mul(wf4, omt, t128)                       # t*(1-t)
    nc.vector.tensor_scalar_mul(wf4, wf4, 4.0)                 # 4t(1-t)

    # ---------------- columns of dev/x0/x1/vel; x in column space ------
    p_vt = psmall.tile([128, 4 * BLOC], F32, tag="small")
    nc.tensor.transpose(
        out=p_vt, in_=sb_vecs, identity=ident[0 : 4 * BLOC, 0 : 4 * BLOC]
    )
    vc = work.tile([128, 4 * BLOC], F32R)  # cols: dev | x0 | x1 | vel
    nc.scalar.copy(out=vc, in_=p_vt)
    devc, x0c = vc[:, 0:2], vc[:, 2:4]
    x1c, velc = vc[:, 4:6], vc[:, 6:8]

    # ||v||^2 via Gram matrix + diagonal extraction
    p_vv = psmall.tile([BLOC, BLOC], F32, tag="small")
    nc.tensor.matmul(p_vv, velc, velc, start=True, stop=True)
    svals = work.tile([BLOC, 2], F32)  # col0 = ||dG||^2, col1 = ||v||^2
    vvd = work.tile([BLOC, BLOC], F32)
    nc.vector.tensor_mul(vvd, p_vv, ident[0:BLOC, 0:BLOC])
    nc.vector.reduce_sum(svals[:, 1:2], vvd, axis=AX.X)

    dxc = work.tile([128, BLOC], F32R)
    nc.vector.tensor_sub(dxc, x1c, x0c)
    xc = work.tile([128, BLOC], F32R)
    nc.vector.scalar_tensor_tensor(
        out=xc, in0=dxc, scalar=t128, in1=x0c, op0=OP.mult, op1=OP.add
    )
    xc_last = nc.vector.scalar_tensor_tensor(
        out=xc, in0=devc, scalar=wf4, in1=xc, op0=OP.mult, op1=OP.add
    )
    vc_b = work.tile([128, 4 * BLOC], BF16)
    i_vcb = nc.vector.tensor_copy(vc_b, vc.bitcast(F32))
    velc_b = vc_b[:, 6:8]
    add_dep_helper(i_vcb.ins, xc_last.ins, sync=False, reason="defer cast past x-chain")

    # u columns per H-tile: [m%128, tile, sample]; h = tanh(u + b1)
    p_uc = psmall.tile([128, 2, BLOC], F32, tag="small")
    for t in range(2):
        nc.tensor.matmul(p_uc[:, t, :], w1_r[:, ts(t)], xc, start=True, stop=True)
    h_c = work.tile([128, 2, BLOC], F32)
    for t in range(2):
        nc.scalar.activation(
            out=h_c[:, t, :], in_=p_uc[:, t, :], func=AF.Tanh,
            bias=sb_b1c[:, t : t + 1], scale=1.0,
        )
    d_c = work.tile([128, 2, BLOC], F32)
    nc.vector.tensor_mul(d_c, h_c, h_c)
    nc.vector.tensor_scalar(
        out=d_c, in0=d_c, scalar1=-1.0, scalar2=1.0, op0=OP.mult, op1=OP.add
    )
    ehd_c = work.tile([128, 2, BLOC, 2], F32R)  # [...,0]=e, [...,1]=h*d
    nc.vector.tensor_mul(ehd_c[:, :, :, 1], h_c, d_c)
    nc.vector.tensor_scalar_mul(ehd_c[:, :, :, 0], ehd_c[:, :, :, 1], -2.0)
    e_c = ehd_c[:, :, :, 0]
    hd_c = ehd_c[:, :, :, 1]
    ehd_b = work.tile([128, 2, BLOC, 2], BF16)
    nc.vector.tensor_copy(ehd_b, ehd_c.bitcast(F32))

    # weights: bf16 for everything but the tanh-input matmul
    # (ordered after the critical x-chain so the DVE FIFO stays clear)
    w1_b = consts.tile([128, H], BF16)
    i_w1b = nc.vector.tensor_copy(w1_b, sb_w1)
    add_dep_helper(i_w1b.ins, xc_last.ins, sync=False, reason="defer cast past x-chain")
    w2_b = consts.tile([128, 2, 128], BF16)
    i_w2b = nc.vector.tensor_copy(w2_b, sb_w2)
    add_dep_helper(i_w2b.ins, xc_last.ins, sync=False, reason="defer cast past x-chain")

    # W2^T and W1^T blocks
    w2t_b = consts.tile([128, H], BF16)
    p_w2t = pbig.tile([128, 512], F32, tag="big")
    for t in range(2):
        nc.tensor.transpose(out=p_w2t[:, ts(t)], in_=sb_w2[:, t, :], identity=ident)
    nc.vector.tensor_copy(w2t_b, p_w2t[:, 0:256])

    w1t_b = consts.tile([128, H], BF16)  # block t = (W1[:, t*128:+128])^T
    p_w1t = pbig.tile([128, 512], F32, tag="big")
    for t in range(2):
        nc.tensor.transpose(out=p_w1t[:, ts(t)], in_=sb_w1[:, ts(t)], identity=ident)
    nc.vector.tensor_copy(w1t_b, p_w1t[:, 0:256])

    # K = W1^T W1, L = W2 W2^T (bf16), KL = K.*L
    kb = consts.tile([128, 2, H], BF16)
    klb = consts.tile([128, 2, H], BF16)
    lb = consts.tile([128, 2, H], BF16)
    for t in range(2):
        p_k = pbig.tile([128, 512], F32, tag="big")
        nc.tensor.matmul(p_k[:, 0:256], w1_b[:, ts(t)], w1_b, start=True, stop=True)
        nc.scalar.copy(out=kb[:, t, :], in_=p_k[:, 0:256])
    for t in range(2):
        p_l = pbig.tile([128, 512], F32, tag="big")
        nc.tensor.matmul(p_l[:, 0:256], w2t_b[:, ts(t)], w2t_b, start=True, stop=True)
        nc.vector.tensor_copy(lb[:, t, :], p_l[:, 0:256])
    for t in range(2):
        nc.vector.tensor_mul(klb[:, t, :], kb[:, t, :], lb[:, t, :])

    # ---------------- S1/S2 small matvecs (both samples batched) -------
    p_gw = psmall.tile([128, 2, 2, BLOC], F32, tag="small")  # [t, {g,w}, s]
    for t in range(2):
        nc.tensor.matmul(p_gw[:, t, 0, :], w2t_b[:, ts(t)], velc_b, start=True, stop=True)
        nc.tensor.matmul(p_gw[:, t, 1, :], w1_b[:, ts(t)], velc_b, start=True, stop=True)
    p_g = p_gw[:, :, 0, :]
    p_w = p_gw[:, :, 1, :]
    dgy = work.tile([128, 2, 2, BLOC], BF16)  # [qt, {dg,yv}, s]
    eg = work.tile([128, 2, BLOC], F32)
    nc.vector.tensor_mul(dgy[:, :, 0, :], p_g, d_c)          # d.*g
    nc.vector.tensor_mul(eg, p_g, e_c)                       # e.*g
    nc.vector.tensor_mul(dgy[:, :, 1, :], eg, p_w)           # e.*g.*w
    p_cvky = psmall.tile([128, 2, 2, BLOC], F32, tag="small")  # [mt, {cv,ky}, s]
    for mt in range(2):
        for qt in range(2):
            nc.tensor.matmul(
                p_cvky[:, mt, :, :], kb[:, qt, ts(mt)], dgy[:, qt, :, :],
                start=(qt == 0), stop=(qt == 1),
            )
    p_cv = p_cvky[:, :, 0, :]
    p_ky = p_cvky[:, :, 1, :]
    z2 = work.tile([128, 2, BLOC], BF16)
    nc.vector.tensor_mul(z2, eg, p_cv)            # e.*g.*cv
    i1 = work.tile([128, 2, BLOC], F32)
    nc.vector.tensor_mul(i1, e_c, p_cv)           # e.*cv
    nc.vector.tensor_mul(i1, i1, p_w)             # e.*cv.*w
    i2 = work.tile([128, 2, BLOC], F32)
    nc.vector.tensor_mul(i2, d_c, p_ky)           # d.*(K y)
    inner = work.tile([128, 2, BLOC], BF16)
    nc.vector.tensor_add(inner, i1, i2)

    # S1 rows -> psum[:, 0:128], 0.5*S2 rows -> psum[:, 128:256]
    p_s12 = prow.tile([BLOC, 256], F32, tag="rows")
    for qt in range(2):
        nc.tensor.matmul(
            p_s12[:, 0:128], inner[:, qt, :], w2_b[:, qt, :],
            start=(qt == 0), stop=(qt == 1),
        )
    for qt in range(2):
        nc.tensor.matmul(
            p_s12[:, 128:256], z2[:, qt, :], w1t_b[:, ts(qt)],
            start=(qt == 0), stop=(qt == 1),
        )

    rest = work.tile([BLOC, N], F32)
    nc.scalar.mul(out=rest, in_=dev_rows, mul=-0.1)

    # ---------------- per-sample norm path (bf16 matmuls) --------------
    # nrm^2 = 8 * (h.*d)^T F (h.*d),  F = (K.*L).*U + K.*V.*W
    acc_cols = work.tile([128, BLOC], F32R)
    # Kd for both samples: [qt, s, 256] so V's rhs is [128, 512]
    kd2 = work.tile([128, 2, BLOC, H], BF16)
    for t in range(2):
        for s in range(BLOC):
            nc.vector.tensor_scalar_mul(
                kd2[:, t, s, :], kb[:, t, :], d_c[:, t, s : s + 1]
            )
    # V = L diag(d) K for both samples: psum [128, s*256 | s*256+256] per pt
    p_vs = []
    for pt in range(2):
        p_v = pbig.tile([128, 512], F32, tag="big")
        for qt in range(2):
            nc.tensor.matmul(
                p_v, lb[:, qt, ts(pt)], kd2[:, qt, :, :],
                start=(qt == 0), stop=(qt == 1),
            )
        p_vs.append(p_v)
    for s in range(BLOC):
        # per-sample V in SBUF; vk = V_s .* K (gpsimd)
        v_sb = loop.tile([128, 2, H], BF16, tag="v_sb")
        vk = loop.tile([128, 2, H], BF16, tag="vk")
        for qt in range(2):
            nc.scalar.copy(out=v_sb[:, qt, :], in_=p_vs[qt][:, s * H : (s + 1) * H])
        for qt in range(2):
            nc.gpsimd.tensor_mul(vk[:, qt, :], v_sb[:, qt, :], kb[:, qt, :])
        # W = K diag(d) L ; U = K diag(d) V  (separate rhs, samples decoupled)
        qs, rs = [], []
        for pt in range(2):
            p_wu = pbig.tile([128, 512], F32, tag="big")
            for qt in range(2):
                nc.tensor.matmul(
                    p_wu[:, 0:H], kd2[:, qt, s, ts(pt)], lb[:, qt, :],
                    start=(qt == 0), stop=(qt == 1),
                )
            for qt in range(2):
                nc.tensor.matmul(
                    p_wu[:, H : 2 * H], kd2[:, qt, s, ts(pt)], v_sb[:, qt, :],
                    start=(qt == 0), stop=(qt == 1),
                )
            q_sb = loop.tile([128, H], BF16, tag="q_sb")
            nc.vector.tensor_mul(q_sb, p_wu[:, H : 2 * H], klb[:, pt, :])
            r_sb = loop.tile([128, H], BF16, tag="r_sb")
            nc.vector.tensor_mul(r_sb, vk[:, pt, :], p_wu[:, 0:H])
            qs.append(q_sb)
            rs.append(r_sb)
        # matvec F (h.*d) distributed over the 4 partial matrices
        p_f = psmall.tile([128, 2, 2], F32, tag="small")  # [mt, {e,hd}]
        for mt in range(2):
            idx = 0
            for qt in range(2):
                for mat in (qs[qt], rs[qt]):
                    nc.tensor.matmul(
                        p_f[:, mt, :], mat[:, ts(mt)], ehd_b[:, qt, s, :],
                        start=(idx == 0), stop=(idx == 3),
                    )
                    idx += 1
        scr = loop.tile([128, 2], F32, tag="scr")
        nc.vector.tensor_mul(scr, p_f[:, :, 1], hd_c[:, :, s])
        with nc.allow_low_precision("f32r accum rounding ~1e-6, fine here"):
            nc.vector.reduce_sum(acc_cols[:, s : s + 1], scr, axis=AX.X)

    # ---------------- final scalars & output ----------------
    p_sc = psmall.tile([BLOC, 2], F32, tag="small")
    nc.tensor.matmul(p_sc, acc_cols, ones2_r, start=True, stop=True)
    nc.scalar.mul(out=svals[:, 0:1], in_=p_sc[:, 0:1], mul=8.0)  # ||dG||^2

    # sqrt via bit trick + Newton (avoids 2nd ACT table load)
    y = work.tile([BLOC, 2], F32)
    nc.vector.tensor_scalar(
        out=y.bitcast(I32), in0=svals.bitcast(I32),
        scalar1=1, scalar2=None, op0=OP.arith_shift_right,
    )
    nc.vector.tensor_scalar(
        out=y.bitcast(I32), in0=y.bitcast(I32),
        scalar1=SQRT_MAGIC, scalar2=None, op0=OP.add,
    )
    rcp = work.tile([BLOC, 2], F32)
    qn = work.tile([BLOC, 2], F32)
    for _ in range(1):
        nc.vector.reciprocal(rcp, y)
        nc.vector.tensor_mul(qn, svals, rcp)          # s / y
        nc.vector.tensor_add(y, y, qn)                # y + s/y
        nc.vector.tensor_scalar_mul(y, y, 0.5)        # 0.5*(y + s/y)
    den = work.tile([BLOC, 1], F32)
    nc.vector.tensor_mul(den, y[:, 0:1], y[:, 1:2])
    inv = work.tile([BLOC, 1], F32)
    nc.vector.reciprocal(inv, den)

    # a = (0.5*S2 - S1) * inv ; out_bot = a + (-0.1*dev)
    s2h = work.tile([BLOC, N], F32)
    nc.scalar.copy(out=s2h, in_=p_s12[:, 128:256])
    comb = work.tile([BLOC, N], F32)
    nc.vector.tensor_sub(comb, s2h, p_s12[:, 0:128])
    bot = work.tile([BLOC, N], F32)
    nc.vector.scalar_tensor_tensor(
        out=bot, in0=comb, scalar=inv, in1=rest, op0=OP.mult, op1=OP.add
    )
    nc.sync.dma_start(out=d_out[:, :], in_=bot)


_NC_CACHE = None


def _get_nc():
    global _NC_CACHE
    if _NC_CACHE is None:
        _NC_CACHE = build_nc()
    return _NC_CACHE


def make_in_maps(inputs):
    """Shard full inputs into per-core input maps."""
    state = np.ascontiguousarray(np.asarray(inputs["state_batch"], dtype=np.float32))
    x0 = np.asarray(inputs["x0_batch"], dtype=np.float32)
    x1 = np.asarray(inputs["x1_batch"], dtype=np.float32)
    W1 = np.ascontiguousarray(np.asarray(inputs["W1"], dtype=np.float32))
    W2 = np.ascontiguousarray(np.asarray(inputs["W2"], dtype=np.float32))
    b1 = np.asarray(inputs["b1"], dtype=np.float32)
    t = np.float32(np.asarray(inputs["t"]).reshape(()))
    dev, vel = state[:B], state[B:]
    misc = np.zeros((128, 4), np.float32)
    misc[:, 0] = t
    misc[:, 1:3] = b1.reshape(2, 128).T
    misc = np.ascontiguousarray(misc)
    in_maps = []
    for c in range(NCORES):
        sl = slice(c * BLOC, (c + 1) * BLOC)
        vecs = np.concatenate([dev[sl], x0[sl], x1[sl], vel[sl]], axis=0)
        in_maps.append(
            {
                "vecs": np.ascontiguousarray(vecs),
                "W1": W1,
                "W2": W2,
                "misc": misc,
            }
        )
    return in_maps, vel


def kernel(**inputs) -> np.ndarray:
    from concourse.bass_utils import run_bass_kernel_spmd

    nc = _get_nc()
    in_maps, vel = make_in_maps(inputs)
    res = run_bass_kernel_spmd(nc, in_maps, core_ids=list(range(NCORES)))
    bottom = np.concatenate([res.results[c]["out_bot"] for c in range(NCORES)], axis=0)
    return np.concatenate([vel, bottom], axis=0).astype(np.float32)


# revision 34
# speedup vs baseline: 1.0445x; 1.0445x over previous
"""Trainium2 Bass kernel for BatchedModelManifoldGeodesicFlow.

Closed-form math (per sample), derived from the reference's autodiff:
  f(x) = tanh(x@W1 + b1)@W2 + b2,  J = jacrev(f)(x) = W2^T diag(d) W1^T
  with h = tanh(x@W1+b1), d = 1-h^2, e = -2*h*d, K = W1^T W1, L = W2 W2^T.
  V := L diag(d) K,  W := K diag(d) L (= V^T),  U := K diag(d) V,
  F := (K.*L).*U + K.*V.*W  (symmetric; uses e = -2*h*d)
    ||dG||^2 = 8 * (h.*d)^T F (h.*d)
  Christoffel contraction -> small matvecs:
    S1 = W2^T [ e.*cv.*w + d.*(K (e.*g.*w)) ],  0.5*S2 = W1 (e.*g.*cv)
    w = W1^T v, g = W2 v, cv = K (d.*g)
    a = (0.5*S2 - S1) / ((||dG||+1e-6) * (||v||+1e-6))
  out = concat([v, a - 0.1*dev], axis=0)

Sharding: pure data parallel, batch 16 -> 2 samples per core on 8 cores.
All heavy matmuls in bf16 (errors cancel statistically in the big norm
sums); the tanh-input matmul stays float32r for accuracy of h.
"""

import sys

if "/opt/trn_rl_repo" not in sys.path:
    sys.path.insert(0, "/opt/trn_rl_repo")

import numpy as np

import concourse.bacc as bacc
import concourse.tile as tile
from concourse import mybir
from concourse.masks import make_identity
from concourse.tile import add_dep_helper

N = 128
H = 256
B = 16
NCORES = 8
BLOC = B // NCORES  # 2 samples per core

F32 = mybir.dt.float32
F32R = mybir.dt.float32r
BF16 = mybir.dt.bfloat16
I32 = mybir.dt.int32
AF = mybir.ActivationFunctionType
OP = mybir.AluOpType
AX = mybir.AxisListType

SQRT_MAGIC = 0x1FBD1DF5  # bits trick: sqrt(x) ~ bitcast((bits(x)>>1) + MAGIC)


def ts(i, sz=128):
    return slice(i * sz, (i + 1) * sz)


def build_nc():
    nc = bacc.Bacc(trn_type="TRN2", enable_partition_id=False)

    # vecs rows: dev(0:2) x0(2:4) x1(4:6) vel(6:8)
    d_vecs = nc.dram_tensor("vecs", [4 * BLOC, N], F32, kind="ExternalInput")
    d_w1 = nc.dram_tensor("W1", [N, H], F32, kind="ExternalInput")
    d_w2 = nc.dram_tensor("W2", [H, N], F32, kind="ExternalInput")
    # misc cols: t128 | b1c (2 cols) | pad
    d_misc = nc.dram_tensor("misc", [128, 4], F32, kind="ExternalInput")
    d_out = nc.dram_tensor("out_bot", [BLOC, N], F32, kind="ExternalOutput")

    with tile.TileContext(nc) as tc:
        with (
            tc.tile_pool(name="consts", bufs=1) as consts,
            tc.tile_pool(name="work", bufs=1) as work,
            tc.tile_pool(name="loop", bufs=2) as loop,
            tc.tile_pool(name="pbig", bufs=4, space="PSUM") as pbig,
            tc.tile_pool(name="psmall", bufs=3, space="PSUM") as psmall,
            tc.tile_pool(name="prow", bufs=1, space="PSUM") as prow,
        ):
            _emit(nc, consts, work, loop, pbig, psmall, prow,
                  d_vecs, d_w1, d_w2, d_misc, d_out)

    nc.compile()  # Bacc: split multi-waits into event semaphores, alloc regs
    return nc


def _emit(nc, consts, work, loop, pbig, psmall, prow,
          d_vecs, d_w1, d_w2, d_misc, d_out):
    # ---------------- input DMA (2 trigger engines in parallel) --------
    sb_w1 = consts.tile([128, H], F32)          # W1[k, m]
    sb_w2 = consts.tile([128, 2, 128], F32)     # W2 tiles: [m%128, m//128, j]
    sb_vecs = consts.tile([4 * BLOC, N], F32)
    sb_misc = consts.tile([128, 4], F32)
    # one trigger per engine so descriptor generation runs in parallel
    nc.sync.dma_start(out=sb_vecs, in_=d_vecs[:, :])
    nc.scalar.dma_start(out=sb_w1, in_=d_w1[:, :])
    nc.gpsimd.dma_start(out=sb_w2, in_=d_w2.rearrange("(t p) n -> p t n", p=128))
    nc.sync.dma_start(out=sb_misc, in_=d_misc[:, :])
    t128 = sb_misc[:, 0:1]
    sb_b1c = sb_misc[:, 1:3]
    dev_rows = sb_vecs[0:BLOC, :]

    # w1_r early: it gates the tanh-input matmul on the critical path
    w1_r = consts.tile([128, H], F32R)
    nc.vector.tensor_copy(w1_r, sb_w1)

    # ---------------- constants ----------------
    ident = consts.tile([128, 128], F32)
    make_identity(nc, ident)
    ones2 = consts.tile([128, 2], F32)
    nc.vector.memset(ones2, 1.0)
    ones2_r = consts.tile([128, 2], F32R)
    nc.vector.tensor_copy(ones2_r, ones2)

    # ---------------- t window: wf4 = 4t(1-t) ----------------
    omt = work.tile([128, 1], F32)
    nc.vector.tensor_scalar(
        out=omt, in0=t128, scalar1=-1.0, scalar2=1.0, op0=OP.mult, op1=OP.add
    )
    wf4 = work.tile([128, 1], F32)
    nc.vector.tensor_mul(wf4, omt, t128)                       # t*(1-t)
    nc.vector.tensor_scalar_mul(wf4, wf4, 4.0)                 # 4t(1-t)

    # ---------------- columns of dev/x0/x1/vel; x in column space ------
    p_vt = psmall.tile([128, 4 * BLOC], F32, tag="small")
    nc.tensor.transpose(
        out=p_vt, in_=sb_vecs, identity=ident[0 : 4 * BLOC, 0 : 4 * BLOC]
    )
    vc = work.tile([128, 4 * BLOC], F32R)  # cols: dev | x0 | x1 | vel
    nc.scalar.copy(out=vc, in_=p_vt)
    devc, x0c = vc[:, 0:2], vc[:, 2:4]
    x1c, velc = vc[:, 4:6], vc[:, 6:8]

    # ||v||^2 via Gram matrix + diagonal extraction
    p_vv = psmall.tile([BLOC, BLOC], F32, tag="small")
    nc.tensor.matmul(p_vv, velc, velc, start=True, stop=True)
    svals = work.tile([BLOC, 2], F32)  # col0 = ||dG||^2, col1 = ||v||^2
    vvd = work.tile([BLOC, BLOC], F32)
    nc.vector.tensor_mul(vvd, p_vv, ident[0:BLOC, 0:BLOC])
    nc.vector.reduce_sum(svals[:, 1:2], vvd, axis=AX.X)

    dxc = work.tile([128, BLOC], F32R)
    nc.vector.tensor_sub(dxc, x1c, x0c)
    xc = work.tile([128, BLOC], F32R)
    nc.vector.scalar_tensor_tensor(
        out=xc, in0=dxc, scalar=t128, in1=x0c, op0=OP.mult, op1=OP.add
    )
    xc_last = nc.vector.scalar_tensor_tensor(
        out=xc, in0=devc, scalar=wf4, in1=xc, op0=OP.mult, op1=OP.add
    )
    vc_b = work.tile([128, 4 * BLOC], BF16)
    i_vcb = nc.vector.tensor_copy(vc_b, vc.bitcast(F32))
    velc_b = vc_b[:, 6:8]
    add_dep_helper(i_vcb.ins, xc_last.ins, sync=False, reason="defer cast past x-chain")

    # u columns per H-tile: [m%128, tile, sample]; h = tanh(u + b1)
    p_uc = psmall.tile([128, 2, BLOC], F32, tag="small")
    for t in range(2):
        nc.tensor.matmul(p_uc[:, t, :], w1_r[:, ts(t)], xc, start=True, stop=True)
    h_c = work.tile([128, 2, BLOC], F32)
    for t in range(2):
        nc.scalar.activation(
            out=h_c[:, t, :], in_=p_uc[:, t, :], func=AF.Tanh,
            bias=sb_b1c[:, t : t + 1], scale=1.0,
        )
    d_c = work.tile([128, 2, BLOC], F32)
    nc.vector.tensor_mul(d_c, h_c, h_c)
    nc.vector.tensor_scalar(
        out=d_c, in0=d_c, scalar1=-1.0, scalar2=1.0, op0=OP.mult, op1=OP.add
    )
    ehd_c = work.tile([128, 2, BLOC, 2], F32R)  # [...,0]=e, [...,1]=h*d
    nc.vector.tensor_mul(ehd_c[:, :, :, 1], h_c, d_c)
    nc.vector.tensor_scalar_mul(ehd_c[:, :, :, 0], ehd_c[:, :, :, 1], -2.0)
    e_c = ehd_c[:, :, :, 0]
    hd_c = ehd_c[:, :, :, 1]
    ehd_b = work.tile([128, 2, BLOC, 2], BF16)
    nc.vector.tensor_copy(ehd_b, ehd_c.bitcast(F32))

    # weights: bf16 for everything but the tanh-input matmul
    # (ordered after the critical x-chain so the DVE FIFO stays clear)
    w1_b = consts.tile([128, H], BF16)
    i_w1b = nc.vector.tensor_copy(w1_b, sb_w1)
    add_dep_helper(i_w1b.ins, xc_last.ins, sync=False, reason="defer cast past x-chain")
    w2_b = consts.tile([128, 2, 128], BF16)
    i_w2b = nc.vector.tensor_copy(w2_b, sb_w2)
    add_dep_helper(i_w2b.ins, xc_last.ins, sync=False, reason="defer cast past x-chain")

    # W2^T and W1^T blocks
    w2t_b = consts.tile([128, H], BF16)
    p_w2t = pbig.tile([128, 512], F32, tag="big")
    for t in range(2):
        nc.tensor.transpose(out=p_w2t[:, ts(t)], in_=sb_w2[:, t, :], identity=ident)
    nc.vector.tensor_copy(w2t_b, p_w2t[:, 0:256])

    w1t_b = consts.tile([128, H], BF16)  # block t = (W1[:, t*128:+128])^T
    p_w1t = pbig.tile([128, 512], F32, tag="big")
    for t in range(2):
        nc.tensor.transpose(out=p_w1t[:, ts(t)], in_=sb_w1[:, ts(t)], identity=ident)
    nc.vector.tensor_copy(w1t_b, p_w1t[:, 0:256])

    # K = W1^T W1, L = W2 W2^T (bf16), KL = K.*L
    kb = consts.tile([128, 2, H], BF16)
    klb = consts.tile([128, 2, H], BF16)
    lb = consts.tile([128, 2, H], BF16)
    for t in range(2):
        p_k = pbig.tile([128, 512], F32, tag="big")
        nc.tensor.matmul(p_k[:, 0:256], w1_b[:, ts(t)], w1_b, start=True, stop=True)
        nc.scalar.copy(out=kb[:, t, :], in_=p_k[:, 0:256])
    for t in range(2):
        p_l = pbig.tile([128, 512], F32, tag="big")
        nc.tensor.matmul(p_l[:, 0:256], w2t_b[:, ts(t)], w2t_b, start=True, stop=True)
        nc.vector.tensor_copy(lb[:, t, :], p_l[:, 0:256])
    for t in range(2):
        nc.vector.tensor_mul(klb[:, t, :], kb[:, t, :], lb[:, t, :])

    # ---------------- S1/S2 small matvecs (both samples batched) -------
    p_gw = psmall.tile([128, 2, 2, BLOC], F32, tag="small")  # [t, {g,w}, s]
    for t in range(2):
        nc.tensor.matmul(p_gw[:, t, 0, :], w2t_b[:, ts(t)], velc_b, start=True, stop=True)
        nc.tensor.matmul(p_gw[:, t, 1, :], w1_b[:, ts(t)], velc_b, start=True, stop=True)
    p_g = p_gw[:, :, 0, :]
    p_w = p_gw[:, :, 1, :]
    dgy = work.tile([128, 2, 2, BLOC], BF16)  # [qt, {dg,yv}, s]
    eg = work.tile([128, 2, BLOC], F32)
    nc.vector.tensor_mul(dgy[:, :, 0, :], p_g, d_c)          # d.*g
    nc.vector.tensor_mul(eg, p_g, e_c)                       # e.*g
    nc.vector.tensor_mul(dgy[:, :, 1, :], eg, p_w)           # e.*g.*w
    p_cvky = psmall.tile([128, 2, 2, BLOC], F32, tag="small")  # [mt, {cv,ky}, s]
    for mt in range(2):
        for qt in range(2):
            nc.tensor.matmul(
                p_cvky[:, mt, :, :], kb[:, qt, ts(mt)], dgy[:, qt, :, :],
                start=(qt == 0), stop=(qt == 1),
            )
    p_cv = p_cvky[:, :, 0, :]
    p_ky = p_cvky[:, :, 1, :]
    z2 = work.tile([128, 2, BLOC], BF16)
    nc.vector.tensor_mul(z2, eg, p_cv)            # e.*g.*cv
    i1 = work.tile([128, 2, BLOC], F32)
    nc.vector.tensor_mul(i1, e_c, p_cv)           # e.*cv
    nc.vector.tensor_mul(i1, i1, p_w)             # e.*cv.*w
    i2 = work.tile([128, 2, BLOC], F32)
    nc.vector.tensor_mul(i2, d_c, p_ky)           # d.*(K y)
    inner = work.tile([128, 2, BLOC], BF16)
    nc.vector.tensor_add(inner, i1, i2)

    # S1 rows -> psum[:, 0:128], 0.5*S2 rows -> psum[:, 128:256]
    p_s12 = prow.tile([BLOC, 256], F32, tag="rows")
    for qt in range(2):
        nc.tensor.matmul(
            p_s12[:, 0:128], inner[:, qt, :], w2_b[:, qt, :],
            start=(qt == 0), stop=(qt == 1),
        )
    for qt in range(2):
        nc.tensor.matmul(
            p_s12[:, 128:256], z2[:, qt, :], w1t_b[:, ts(qt)],
            start=(qt == 0), stop=(qt == 1),
        )

    rest = work.tile([BLOC, N], F32)
    nc.scalar.mul(out=rest, in_=dev_rows, mul=-0.1)

    # ---------------- per-sample norm path (bf16 matmuls) --------------
    # nrm^2 = 8 * (h.*d)^T F (h.*d),  F = (K.*L).*U + K.*V.*W
    acc_cols = work.tile([128, BLOC], F32R)
    # Kd for both samples: [qt, s, 256] so V's rhs is [128, 512]
    kd2 = work.tile([128, 2, BLOC, H], BF16)
    for t in range(2):
        for s in range(BLOC):
            nc.vector.tensor_scalar_mul(
                kd2[:, t, s, :], kb[:, t, :], d_c[:, t, s : s + 1]
            )
    # V = L diag(d) K for both samples: psum [128, s*256 | s*256+256] per pt
    p_vs = []
    for pt in range(2):
        p_v = pbig.tile([128, 512], F32, tag="big")
        for qt in range(2):
            nc.tensor.matmul(
                p_v, lb[:, qt, ts(pt)], kd2[:, qt, :, :],
                start=(qt == 0), stop=(qt == 1),
            )
        p_vs.append(p_v)
    for s in range(BLOC):
        # per-sample V in SBUF; vk = V_s .* K (gpsimd)
        v_sb = loop.tile([128, 2, H], BF16, tag="v_sb")
        vk = loop.tile([128, 2, H], BF16, tag="vk")
        for qt in range(2):
            nc.scalar.copy(out=v_sb[:, qt, :], in_=p_vs[qt][:, s * H : (s + 1) * H])
        for qt in range(2):
            nc.gpsimd.tensor_mul(vk[:, qt, :], v_sb[:, qt, :], kb[:, qt, :])
        # W = K diag(d) L ; U = K diag(d) V  (separate rhs, samples decoupled)
        qs, rs = [], []
        for pt in range(2):
            p_wu = pbig.tile([128, 512], F32, tag="big")
            for qt in range(2):
                nc.tensor.matmul(
                    p_wu[:, 0:H], kd2[:, qt, s, ts(pt)], lb[:, qt, :],
                    start=(qt == 0), stop=(qt == 1),
                )
            for qt in range(2):
                nc.tensor.matmul(
                    p_wu[:, H : 2 * H], kd2[:, qt, s, ts(pt)], v_sb[:, qt, :],
                    start=(qt == 0), stop=(qt == 1),
                )
            q_sb = loop.tile([128, H], BF16, tag="q_sb")
            nc.vector.tensor_mul(q_sb, p_wu[:, H : 2 * H], klb[:, pt, :])
            r_sb = loop.tile([128, H], BF16, tag="r_sb")
            nc.vector.tensor_mul(r_sb, vk[:, pt, :], p_wu[:, 0:H])
            qs.append(q_sb)
            rs.append(r_sb)
        # matvec F (h.*d) distributed over the 4 partial matrices
        p_f = psmall.tile([128, 2, 2], F32, tag="small")  # [mt, {e,hd}]
        for mt in range(2):
            idx = 0
            for qt in range(2):
                for mat in (qs[qt], rs[qt]):
                    nc.tensor.matmul(
                        p_f[:, mt, :], mat[:, ts(mt)], ehd_b[:, qt, s, :],
                        start=(idx == 0), stop=(idx == 3),
                    )
                    idx += 1
        scr = loop.tile([128, 2], F32, tag="scr")
        nc.vector.tensor_mul(scr, p_f[:, :, 1], hd_c[:, :, s])
        with nc.allow_low_precision("f32r accum rounding ~1e-6, fine here"):
            nc.vector.reduce_sum(acc_cols[:, s : s + 1], scr, axis=AX.X)

    # ---------------- final scalars & output ----------------
    p_sc = psmall.tile([BLOC, 2], F32, tag="small")
    nc.tensor.matmul(p_sc, acc_cols, ones2_r, start=True, stop=True)
    nc.scalar.mul(out=svals[:, 0:1], in_=p_sc[:, 0:1], mul=8.0)  # ||dG||^2

    # sqrt via bit trick + Newton (avoids 2nd ACT table load)
    y = work.tile([BLOC, 2], F32)
    nc.vector.tensor_scalar(
        out=y.bitcast(I32), in0=svals.bitcast(I32),
        scalar1=1, scalar2=None, op0=OP.arith_shift_right,
    )
    nc.vector.tensor_scalar(
        out=y.bitcast(I32), in0=y.bitcast(I32),
        scalar1=SQRT_MAGIC, scalar2=None, op0=OP.add,
    )
    rcp = work.tile([BLOC, 2], F32)
    qn = work.tile([BLOC, 2], F32)
    for _ in range(1):
        nc.vector.reciprocal(rcp, y)
        nc.vector.tensor_mul(qn, svals, rcp)          # s / y
        nc.vector.tensor_add(y, y, qn)                # y + s/y
        nc.vector.tensor_scalar_mul(y, y, 0.5)        # 0.5*(y + s/y)
    den = work.tile([BLOC, 1], F32)
    nc.vector.tensor_mul(den, y[:, 0:1], y[:, 1:2])
    inv = work.tile([BLOC, 1], F32)
    nc.vector.reciprocal(inv, den)

    # a = (0.5*S2 - S1) * inv ; out_bot = a + (-0.1*dev)
    s2h = work.tile([BLOC, N], F32)
    nc.scalar.copy(out=s2h, in_=p_s12[:, 128:256])
    comb = work.tile([BLOC, N], F32)
    nc.vector.tensor_sub(comb, s2h, p_s12[:, 0:128])
    bot = work.tile([BLOC, N], F32)
    nc.vector.scalar_tensor_tensor(
        out=bot, in0=comb, scalar=inv, in1=rest, op0=OP.mult, op1=OP.add
    )
    nc.sync.dma_start(out=d_out[:, :], in_=bot)


_NC_CACHE = None


def _get_nc():
    global _NC_CACHE
    if _NC_CACHE is None:
        _NC_CACHE = build_nc()
    return _NC_CACHE


def make_in_maps(inputs):
    """Shard full inputs into per-core input maps."""
    state = np.ascontiguousarray(np.asarray(inputs["state_batch"], dtype=np.float32))
    x0 = np.asarray(inputs["x0_batch"], dtype=np.float32)
    x1 = np.asarray(inputs["x1_batch"], dtype=np.float32)
    W1 = np.ascontiguousarray(np.asarray(inputs["W1"], dtype=np.float32))
    W2 = np.ascontiguousarray(np.asarray(inputs["W2"], dtype=np.float32))
    b1 = np.asarray(inputs["b1"], dtype=np.float32)
    t = np.float32(np.asarray(inputs["t"]).reshape(()))
    dev, vel = state[:B], state[B:]
    misc = np.zeros((128, 4), np.float32)
    misc[:, 0] = t
    misc[:, 1:3] = b1.reshape(2, 128).T
    misc = np.ascontiguousarray(misc)
    in_maps = []
    for c in range(NCORES):
        sl = slice(c * BLOC, (c + 1) * BLOC)
        vecs = np.concatenate([dev[sl], x0[sl], x1[sl], vel[sl]], axis=0)
        in_maps.append(
            {
                "vecs": np.ascontiguousarray(vecs),
                "W1": W1,
                "W2": W2,
                "misc": misc,
            }
        )
    return in_maps, vel


def kernel(**inputs) -> np.ndarray:
    from concourse.bass_utils import run_bass_kernel_spmd

    nc = _get_nc()
    in_maps, vel = make_in_maps(inputs)
    res = run_bass_kernel_spmd(nc, in_maps, core_ids=list(range(NCORES)))
    bottom = np.concatenate([res.results[c]["out_bot"] for c in range(NCORES)], axis=0)
    return np.concatenate([vel, bottom], axis=0).astype(np.float32)
